# revision 14
# baseline (speedup 1.0000x reference)
"""Trainium2 Bass kernel for 3-layer GAT + global_add_pool + linear head.

Design (v2):
- Nodes (and incoming edges) sharded across 8 cores by dst.
- Node phase per layer: h_aug = x @ [W | W@As | W@Ad] on PE (bf16 in, f32 out).
  [h|alpha_src] rows (bf16, 512B stride) go to a DRAM table that is
  AllGathered; alpha_dst rows (f32, 256B stride) go to a core-local table.
- Edge phase: edges are tiled 128/dst-block; per chunk of CT tiles, batched
  InstDMAGatherAnt instructions fetch h[src] rows (two streams: lo/hi table
  half, int16 idx limit) and alpha_dst[dst] rows (local table, also carries
  the dst-slot value used to build selector matrices).
- e = lrelu(a_s+a_d) computed on vector (max(z,.2z)), exp batched on scalar,
  messages scaled in place (bf16), then per-tile selector matmuls accumulate
  [w*h | w] into PSUM per dst block; normalization + relu on block end.
- Partial pooled logits summed on host.

Self-contained: no file reads; shapes hardcoded via constants.
"""
import math
import numpy as np
from contextlib import ExitStack

import concourse.bass as bass
import concourse.mybir as mybir
import concourse.tile as tile
from concourse.bass_utils import run_bass_kernel_spmd
from concourse.tile_rust import add_dep_helper
from concourse import library_config

NCORES = 8
P = 128
H = 4
Ch = 32
HC = 128
AUGW = HC + 2 * H   # 136: node matmul out [h | a_src | a_dst]
TBL = HC + H        # 132: useful row [h | a_src]
TW = 256            # bf16 table row width (512B stride)
AW = 64             # f32 alpha_dst table row width (256B stride)
NEG_SLOPE = 0.2
GRAPHS = 64
OUT = 10
CT = 32             # tiles (of 128 edges) per chunk
import os as _os
K_STAGE = int(_os.environ.get("K_STAGE", "9"))
# 1: node+AG only; 2: +gathers; 3: +vector chunk ops; 4: +agg matmuls; 9: full

# instruction types whose BIR struct cannot carry all Tile-emitted waits
_WAIT_CAPS = {
    "InstDMAGatherAnt": 0,
    "InstDMAScatterAddAnt": 0,
    "InstNoOp": 1,
    "InstDrain": 1,
    "InstCollectiveCompute": 1,
}


def _fixup_wait_limits(nc):
    k = 0
    for fn in nc.m.functions:
        for blk in fn.blocks:
            out = []
            for inst in blk.instructions:
                cap = _WAIT_CAPS.get(type(inst).__name__, 1)
                si = inst.sync_info
                if si is not None:
                    waits = list(si.on_wait)
                    if len(waits) > cap:
                        keep, move = waits[:cap], waits[cap:]
                        for w in move:
                            nop = mybir.InstNoOp(name=f"waitfix_{k}", text_hint="wait_fixup")
                            k += 1
                            nop.engine = inst.engine
                            nop.sync_info = type(si)(on_wait=[w], on_update=[])
                            out.append(nop)
                        inst.sync_info = type(si)(on_wait=list(keep), on_update=list(si.on_update))
                out.append(inst)
            blk.instructions = out
    return k


def _prep_edges(src_all, dst_all, per, nb, npad):
    """Tile edges per core by (dst block, src half), build gather idx arrays.

    Returns (blk_of_tile, start_t, stop_t, bufcol, n_lo, Tpad, srcidx, adidx)
    where srcidx/adidx are per-core [128, Tpad*8] int16 SBUF images.
    """
    NHALF = npad // 2
    percore = []
    cnt_bh = np.zeros((nb, 2), np.int64)
    for c in range(NCORES):
        m = (dst_all // per) == c
        s = src_all[m]
        loc = dst_all[m] - c * per
        b = loc // P
        hf = (s >= NHALF).astype(np.int64)
        order = np.lexsort((hf, b))
        s, loc, b, hf = s[order], loc[order], b[order], hf[order]
        cnt = np.bincount(b * 2 + hf, minlength=nb * 2).reshape(nb, 2)
        cnt_bh = np.maximum(cnt_bh, cnt)
        percore.append((s, loc, b, hf, cnt))

    tiles_bh = (cnt_bh + P - 1) // P            # [nb, 2]
    # processing-order tiles
    blk_l, hf_l = [], []
    tstart = np.zeros((nb, 2), np.int64)
    t = 0
    for b in range(nb):
        for hf in (0, 1):
            tstart[b, hf] = t
            n = int(tiles_bh[b, hf])
            blk_l += [b] * n
            hf_l += [hf] * n
            t += n
    T = t
    nchunks = (T + CT - 1) // CT
    Tpad = nchunks * CT
    blk_of_tile = np.array(blk_l + [nb - 1] * (Tpad - T), np.int64)
    hf_of_tile = np.array(hf_l + [0] * (Tpad - T), np.int64)
    start_t = np.zeros(Tpad, bool)
    stop_t = np.zeros(Tpad, bool)
    for b in range(nb):
        w = np.nonzero(blk_of_tile == b)[0]
        start_t[w[0]] = True
        stop_t[w[-1]] = True

    # buffer-column mapping: per chunk, lo tiles first then hi tiles
    bufcol = np.zeros(Tpad, np.int64)
    n_lo = []
    for ch in range(nchunks):
        ts = np.arange(ch * CT, (ch + 1) * CT)
        lo = ts[hf_of_tile[ts] == 0]
        hi = ts[hf_of_tile[ts] == 1]
        bufcol[lo] = np.arange(len(lo))
        bufcol[hi] = len(lo) + np.arange(len(hi))
        n_lo.append(len(lo))

    SW = Tpad * 8
    srcidxs, adidxs = [], []
    for c in range(NCORES):
        s, loc, b, hf, cnt = percore[c]
        srcv = np.zeros((P, Tpad), np.int64)
        adv = np.full((P, Tpad), per, np.int64)   # pad slots -> pad row
        off = np.zeros(nb * 2 + 1, np.int64)
        off[1:] = np.cumsum(cnt.reshape(-1))
        key = b * 2 + hf
        pos = np.arange(len(s)) - off[key]
        tt = tstart[b, hf] + pos // P
        pp = pos % P
        srcv[pp, tt] = s - hf * NHALF
        adv[pp, tt] = loc
        # sbuf idx image: slot (p, t) -> row 16k+p%16, col (chunkbase+bufcol)*8+p//16
        p_g, t_g = np.mgrid[0:P, 0:Tpad]
        col = ((t_g // CT) * CT + bufcol[t_g]) * 8 + p_g // 16
        row = p_g % 16
        si = np.zeros((P, SW), np.int16)
        ai = np.zeros((P, SW), np.int16)
        for k in range(8):
            si[16 * k + row, col] = srcv
            ai[16 * k + row, col] = adv
        srcidxs.append(si)
        adidxs.append(ai)
    return (blk_of_tile, start_t, stop_t, bufcol, n_lo, Tpad, srcidxs, adidxs)


def _build(npad, Tpad, blk_of_tile, start_t, stop_t, bufcol, n_lo):
    per = npad // NCORES
    nb = per // P
    NHALF = npad // 2
    nlayers = 3
    nchunks = Tpad // CT
    f32 = mybir.dt.float32
    bf16 = mybir.dt.bfloat16
    i16 = mybir.dt.int16

    nc = bass.Bass(num_devices=NCORES)
    # ---- dram I/O
    xT_d = nc.dram_tensor("xT", [P, per], bf16, kind="ExternalInput")
    waug_d = nc.dram_tensor("waug", [nlayers, P, AUGW], bf16, kind="ExternalInput")
    wh_d = nc.dram_tensor("wh", [P, OUT], f32, kind="ExternalInput")
    iota_d = nc.dram_tensor("iota", [P, 4 * P], f32, kind="ExternalInput")
    ident_d = nc.dram_tensor("ident", [P, P], f32, kind="ExternalInput")
    SW = Tpad * 8
    srcidx_d = nc.dram_tensor("srcidx", [P, SW], i16, kind="ExternalInput")
    adidx_d = nc.dram_tensor("adidx", [P, SW], i16, kind="ExternalInput")
    batchf_d = nc.dram_tensor("batchf", [P, nb], f32, kind="ExternalInput")
    padrow_d = nc.dram_tensor("padrow", [1, AW], f32, kind="ExternalInput")
    dlocc_d = nc.dram_tensor("dlocc", [P, per // P], f32, kind="ExternalInput")
    out_d = nc.dram_tensor("out", [GRAPHS, OUT], f32, kind="ExternalOutput")

    h_loc = [nc.dram_tensor(f"h_loc{l}", [per, TW], bf16) for l in range(nlayers)]
    h_full = [nc.dram_tensor(f"h_full{l}", [npad, TW], bf16, addr_space="Shared")
              for l in range(nlayers)]
    ad_tbl = [nc.dram_tensor(f"ad_tbl{l}", [per + P, AW], f32)
              for l in range(nlayers)]

    groups = [list(range(NCORES))]

    with ExitStack() as ctx:
        tc = ctx.enter_context(tile.TileContext(nc))
        sb = ctx.enter_context(tc.tile_pool(name="sb", bufs=1))
        sb_g = ctx.enter_context(tc.tile_pool(name="sbg", bufs=2))
        sb_a = ctx.enter_context(tc.tile_pool(name="sba", bufs=2))
        sb_s = ctx.enter_context(tc.tile_pool(name="sbs", bufs=2))
        sb_w = ctx.enter_context(tc.tile_pool(name="sbw", bufs=3))
        ps_h = ctx.enter_context(tc.tile_pool(name="psh", bufs=2, space="PSUM"))
        ps_agg = ctx.enter_context(tc.tile_pool(name="psagg", bufs=2, space="PSUM"))
        ps_xp = ctx.enter_context(tc.tile_pool(name="psxp", bufs=1, space="PSUM"))
        ps_fin = ctx.enter_context(tc.tile_pool(name="psfin", bufs=1, space="PSUM"))

        # ---- persistent SBUF state
        xT = sb.tile([P, per], bf16)
        nc.sync.dma_start(out=xT[:], in_=xT_d[:])
        waug = sb.tile([P, nlayers, AUGW], bf16)
        nc.sync.dma_start(out=waug[:],
                          in_=waug_d[:].rearrange("l p a -> p l a"))
        wh = sb.tile([P, OUT], f32)
        nc.sync.dma_start(out=wh[:], in_=wh_d[:])
        iota = sb.tile([P, 4, P], f32)
        nc.sync.dma_start(out=iota[:].rearrange("p a b -> p (a b)"), in_=iota_d[:])
        srci = sb.tile([P, SW], i16)
        nc.sync.dma_start(out=srci[:], in_=srcidx_d[:])
        adix = sb.tile([P, SW], i16)
        nc.sync.dma_start(out=adix[:], in_=adidx_d[:])
        batchf = sb.tile([P, nb, 1], f32)
        nc.sync.dma_start(out=batchf[:].rearrange("p b o -> p (b o)"), in_=batchf_d[:])
        identf = sb.tile([P, P], f32)
        nc.sync.dma_start(out=identf[:], in_=ident_d[:])
        padsb = sb.tile([1, AW], f32)
        nc.sync.dma_start(out=padsb[:], in_=padrow_d[:])
        dlocsb = sb.tile([P, nb], f32)
        nc.sync.dma_start(out=dlocsb[:], in_=dlocc_d[:])
        # pad row of each layer's alpha_dst table (a_d=0, dloc=-1)
        dpads = [nc.sync.dma_start(out=ad_tbl[l][per:per + 1, :], in_=padsb[:])
                 for l in range(nlayers)]
        # static dst-slot column (col H) of each alpha_dst table
        ddlocs = [nc.sync.dma_start(
            out=ad_tbl[l][0:per, H:H + 1].rearrange("(b p) d -> p b d", p=P),
            in_=dlocsb[:].rearrange("p (b o) -> p b o", o=1))
            for l in range(nlayers)]

        nc.gpsimd.load_library(library_config.mlp)

        _regs = {}

        def nreg(v):
            if v not in _regs:
                _regs[v] = nc.gpsimd.to_reg(v)
            return _regs[v]

        hsb = sb.tile([P, nb, TBL], bf16)
        adsb = sb.tile([P, nb, H], f32)
        pooled_ps = ps_fin.tile([GRAPHS, HC], f32)

        for l in range(3):
            # ===== node phase =====
            for b in range(nb):
                ps = ps_h.tile([P, AUGW], f32, tag="ndps")
                nc.tensor.matmul(ps[:], lhsT=xT[:, b * P:(b + 1) * P],
                                 rhs=waug[:, l, :], start=True, stop=True)
                nc.vector.tensor_copy(out=hsb[:, b, :], in_=ps[:, :TBL])
                nc.vector.tensor_copy(out=adsb[:, b, :], in_=ps[:, TBL:AUGW])
            dh = nc.sync.dma_start(
                out=h_loc[l][:, 0:TBL].rearrange("(b p) d -> p b d", p=P),
                in_=hsb[:])
            da = nc.sync.dma_start(
                out=ad_tbl[l][0:per, 0:H].rearrange("(b p) d -> p b d", p=P),
                in_=adsb[:])
            cch = nc.gpsimd.collective_compute(
                "AllGather", mybir.AluOpType.bypass, replica_groups=groups,
                ins=[h_loc[l][:]], outs=[h_full[l][:]])
            add_dep_helper(cch.ins, dh.ins, sync=True, reason="h write before ag")

            # ===== edge phase =====
            agg_of_blk = {}
            for ci in range(nchunks):
                if K_STAGE < 2:
                    break
                t0 = ci * CT
                nlo = int(n_lo[ci])
                g = sb_g.tile([P, CT, TW], bf16, tag="gath")
                if nlo > 0:
                    glo = nc.gpsimd.dma_gather(
                        out_ap=g[:, 0:nlo, :], in_ap=h_full[l][0:NHALF, :],
                        idxs_ap=srci[:, t0 * 8:t0 * 8 + nlo * 8],
                        num_idxs=nlo * P, num_idxs_reg=nreg(nlo * P), elem_size=TW,
                        single_packet=False)
                    add_dep_helper(glo.ins, cch.ins, sync=True, reason="gather after ag")
                if nlo < CT:
                    ghi = nc.gpsimd.dma_gather(
                        out_ap=g[:, nlo:CT, :], in_ap=h_full[l][NHALF:npad, :],
                        idxs_ap=srci[:, t0 * 8 + nlo * 8:(t0 + CT) * 8],
                        num_idxs=(CT - nlo) * P, num_idxs_reg=nreg((CT - nlo) * P),
                        elem_size=TW, single_packet=False)
                    add_dep_helper(ghi.ins, cch.ins, sync=True, reason="gather after ag")
                a = sb_a.tile([P, CT, AW], f32, tag="adg")
                gad = nc.gpsimd.dma_gather(
                    out_ap=a[:], in_ap=ad_tbl[l][:],
                    idxs_ap=adix[:, t0 * 8:(t0 + CT) * 8],
                    num_idxs=CT * P, num_idxs_reg=nreg(CT * P), elem_size=AW,
                    single_packet=False)
                add_dep_helper(gad.ins, da.ins, sync=True, reason="ad gather after write")
                add_dep_helper(gad.ins, dpads[l].ins, sync=True, reason="ad gather after padrow")
                add_dep_helper(gad.ins, ddlocs[l].ins, sync=True, reason="ad gather after dloc col")

                if K_STAGE < 3:
                    continue
                # e = lrelu(a_s + a_d); w = exp(e)
                lg = sb_w.tile([P, CT, H], f32, tag="lg")
                nc.vector.tensor_tensor(out=lg[:], in0=a[:, :, 0:H],
                                        in1=g[:, :, HC:TBL],
                                        op=mybir.AluOpType.add)
                lr = sb_w.tile([P, CT, H], f32, tag="lr")
                nc.vector.tensor_scalar_mul(lr[:], lg[:], NEG_SLOPE)
                nc.vector.tensor_tensor(out=lr[:], in0=lr[:], in1=lg[:],
                                        op=mybir.AluOpType.max)
                ex = sb_w.tile([P, CT, H], f32, tag="ex")
                nc.scalar.activation(ex[:], lr[:],
                                     mybir.ActivationFunctionType.Exp)
                # msg in place: g[:, :, h*Ch:(h+1)*Ch] *= ex[h]; cols HC:TBL = ex
                for h in range(H):
                    nc.vector.tensor_tensor(
                        out=g[:, :, h * Ch:(h + 1) * Ch],
                        in0=g[:, :, h * Ch:(h + 1) * Ch],
                        in1=ex[:, :, h:h + 1].to_broadcast([P, CT, Ch]),
                        op=mybir.AluOpType.mult)
                nc.vector.tensor_copy(out=g[:, :, HC:TBL], in_=ex[:])
                # selectors for the whole chunk (dst one-hot per edge)
                sel = sb_s.tile([P, CT, P], bf16, tag="sel")
                for q in range(CT // 4):
                    nc.vector.tensor_tensor(
                        out=sel[:, 4 * q:4 * q + 4, :],
                        in0=a[:, 4 * q:4 * q + 4, H:H + 1].to_broadcast([P, 4, P]),
                        in1=iota[:], op=mybir.AluOpType.is_equal)
                if K_STAGE < 4:
                    continue
                # aggregate per tile (processing order)
                for t in range(t0, t0 + CT):
                    b = int(blk_of_tile[t])
                    j = int(bufcol[t])
                    if start_t[t]:
                        agg_of_blk[b] = ps_agg.tile([P, TBL], f32, tag="agg",
                                                    name=f"agg{l}_{b}")
                    nc.tensor.matmul(agg_of_blk[b][:], lhsT=sel[:, j, :],
                                     rhs=g[:, j, 0:TBL],
                                     start=bool(start_t[t]),
                                     stop=bool(stop_t[t]))
                    if stop_t[t]:
                        agg = agg_of_blk.pop(b)
                        if K_STAGE < 9:
                            continue
                        rec = sb_w.tile([P, H], f32, tag="rec")
                        nc.vector.reciprocal(rec[:], agg[:, HC:TBL])
                        xb = sb_w.tile([P, HC], f32, tag="xb")
                        for h in range(H):
                            nc.vector.tensor_tensor(
                                out=xb[:, h * Ch:(h + 1) * Ch],
                                in0=agg[:, h * Ch:(h + 1) * Ch],
                                in1=rec[:, h:h + 1].to_broadcast([P, Ch]),
                                op=mybir.AluOpType.mult)
                        nc.vector.tensor_scalar_max(xb[:], xb[:], 0.0)
                        if l < 2:
                            xps = ps_xp.tile([P, P], f32, tag="xps")
                            nc.tensor.transpose(xps[:], xb[:], identf[:])
                            nc.vector.tensor_copy(
                                out=xT[:, b * P:(b + 1) * P], in_=xps[:])
                        else:
                            bsel = sb_w.tile([P, GRAPHS], f32, tag="bsel")
                            nc.vector.tensor_tensor(
                                out=bsel[:],
                                in0=batchf[:, b, :].to_broadcast([P, GRAPHS]),
                                in1=iota[:, 0, :GRAPHS],
                                op=mybir.AluOpType.is_equal)
                            nc.tensor.matmul(pooled_ps[:], lhsT=bsel[:],
                                             rhs=xb[:], start=(b == 0),
                                             stop=(b == nb - 1))

        # ===== head =====
        pooled_sb = sb.tile([GRAPHS, HC], f32)
        nc.vector.tensor_copy(out=pooled_sb[:], in_=pooled_ps[:])
        pT_ps = ps_xp.tile([P, GRAPHS], f32, tag="xps")
        nc.tensor.transpose(pT_ps[:], pooled_sb[:], identf[:GRAPHS, :GRAPHS])
        pT_sb = sb.tile([P, GRAPHS], f32)
        nc.vector.tensor_copy(out=pT_sb[:], in_=pT_ps[:])
        log_ps = ps_xp.tile([GRAPHS, OUT], f32, tag="logps")
        nc.tensor.matmul(log_ps[:], lhsT=pT_sb[:], rhs=wh[:], start=True, stop=True)
        log_sb = sb.tile([GRAPHS, OUT], f32)
        nc.vector.tensor_copy(out=log_sb[:], in_=log_ps[:])
        nc.sync.dma_start(out=out_d[:], in_=log_sb[:])

    _fixup_wait_limits(nc)
    mybir.codegen_inst_isa_subclasses(nc)
    return nc


def prepare(x, Ws, a_srcs, a_dsts, biases, Wh, bh, edge_index, batch):
    n = x.shape[0]
    npad = int(math.ceil(n / (NCORES * P)) * NCORES * P)
    per = npad // NCORES
    nb = per // P

    x = np.asarray(x, np.float32)
    Ws = [np.asarray(w, np.float32) for w in Ws]
    a_srcs = [np.asarray(a, np.float32) for a in a_srcs]
    a_dsts = [np.asarray(a, np.float32) for a in a_dsts]
    Wh = np.asarray(Wh, np.float32)
    bh = np.asarray(bh, np.float32)
    edge_index = np.asarray(edge_index)
    batch = np.asarray(batch)
    for b in biases:
        assert np.allclose(np.asarray(b), 0.0), "nonzero GAT biases unsupported"

    import ml_dtypes
    # W_aug = [W | W@As | W@Ad]
    waugs = []
    for l in range(3):
        As = np.zeros((HC, H), np.float32)
        Ad = np.zeros((HC, H), np.float32)
        for h in range(H):
            As[h * Ch:(h + 1) * Ch, h] = a_srcs[l][h]
            Ad[h * Ch:(h + 1) * Ch, h] = a_dsts[l][h]
        W = Ws[l]
        waugs.append(np.concatenate([W, W @ As, W @ Ad], axis=1))
    waug = np.stack(waugs, 0).astype(ml_dtypes.bfloat16)  # [3, 128, AUGW]

    # edges + self loops (incl. pad nodes, so every row has >=1 edge)
    src_all = np.concatenate([edge_index[0].astype(np.int64),
                              np.arange(npad, dtype=np.int64)])
    dst_all = np.concatenate([edge_index[1].astype(np.int64),
                              np.arange(npad, dtype=np.int64)])
    (blk_of_tile, start_t, stop_t, bufcol, n_lo, Tpad, srcidxs, adidxs) = \
        _prep_edges(src_all, dst_all, per, nb, npad)

    xpad = np.zeros((npad, HC), np.float32)
    xpad[:n] = x
    iota = np.tile(np.arange(P, dtype=np.float32)[None, :], (P, 4))

    batchf_full = np.full(npad, -1.0, np.float32)
    batchf_full[:n] = batch.astype(np.float32)

    padrow = np.zeros((1, AW), np.float32)
    padrow[0, H] = -1.0     # dloc slot

    nc = _build(npad, Tpad, blk_of_tile, start_t, stop_t, bufcol, n_lo)

    in_maps = []
    for c in range(NCORES):
        sl = slice(c * per, (c + 1) * per)
        in_maps.append({
            "xT": np.ascontiguousarray(xpad[sl].T).astype(ml_dtypes.bfloat16),
            "waug": waug,
            "wh": Wh,
            "iota": iota,
            "srcidx": srcidxs[c],
            "adidx": adidxs[c],
            "batchf": np.ascontiguousarray(
                batchf_full[sl].reshape(nb, P).T),
            "padrow": padrow,
            "ident": np.eye(P, dtype=np.float32),
            "dlocc": np.tile(np.arange(P, dtype=np.float32)[:, None], (1, nb)),
            })
    return nc, in_maps


def run_gat(x, Ws, a_srcs, a_dsts, biases, Wh, bh, edge_index, batch):
    nc, in_maps = prepare(x, Ws, a_srcs, a_dsts, biases, Wh, bh,
                          edge_index, batch)
    res = run_bass_kernel_spmd(nc, in_maps, list(range(NCORES)))
    global LAST_EXEC_NS
    LAST_EXEC_NS = getattr(res, "exec_time_ns", None)
    logits = np.zeros((GRAPHS, OUT), np.float32)
    for c in range(NCORES):
        logits += res.results[c]["out"]
    return logits + bh


def kernel(**inputs):
    return np.asarray(run_gat(
        inputs["x"], inputs["Ws"], inputs["a_srcs"], inputs["a_dsts"],
        inputs["biases"], inputs["Wh"], inputs["bh"], inputs["edge_index"],
        inputs["batch"]), np.float32)


# revision 16
# speedup vs baseline: 1.9513x; 1.9513x over previous
"""Trainium2 Bass kernel for 3-layer GAT + global_add_pool + linear head.

Design (v3):
- Nodes (and incoming edges) sharded across 8 cores by dst.
- Node phase per layer: h_aug = x @ [W | W@As | W@Ad] on PE (bf16 in, f32 out).
  [h|alpha_src] rows (bf16, 512B stride) go to a DRAM table that is
  AllGathered across cores.
- Edge phase: edges tiled 128/dst-block (sub-split by src half for int16
  gather indices); per chunk of CT tiles, batched InstDMAGatherAnt fetches
  h[src] rows on 4 SWDGE queues. Per-edge alpha_dst comes from transposed
  selector matmuls on the (otherwise idle) PE; selectors are built by vector
  is_equal against a streamed dst-slot pattern.
- e = lrelu(a_s+a_d) on vector (max(z,.2z)), exp batched on scalar, messages
  scaled in place (bf16), then per-tile selector matmuls accumulate [w*h | w]
  into PSUM per dst block; normalization + relu on block end.
- Partial pooled logits summed on host.

Self-contained: no file reads; shapes hardcoded via constants.
"""
import math
import numpy as np
from contextlib import ExitStack

import concourse.bass as bass
import concourse.mybir as mybir
import concourse.tile as tile
from concourse.bass_utils import run_bass_kernel_spmd
from concourse.tile_rust import add_dep_helper
from concourse import library_config

NCORES = 8
P = 128
H = 4
Ch = 32
HC = 128
AUGW = HC + 2 * H   # 136: node matmul out [h | a_src | a_dst]
TBL = HC + H        # 132: useful row [h | a_src]
TW = 256            # bf16 table row width (512B stride)
NEG_SLOPE = 0.2
GRAPHS = 64
OUT = 10
CT = 32             # tiles (of 128 edges) per chunk
NSWQ = 4            # SWDGE queues

# instruction types whose BIR struct cannot carry all Tile-emitted waits
_WAIT_CAPS = {
    "InstDMAGatherAnt": 0,
    "InstDMAScatterAddAnt": 0,
    "InstNoOp": 1,
    "InstDrain": 1,
    "InstCollectiveCompute": 1,
}


def _fixup_wait_limits(nc):
    k = 0
    for fn in nc.m.functions:
        for blk in fn.blocks:
            out = []
            for inst in blk.instructions:
                cap = _WAIT_CAPS.get(type(inst).__name__, 1)
                si = inst.sync_info
                if si is not None:
                    waits = list(si.on_wait)
                    if len(waits) > cap:
                        keep, move = waits[:cap], waits[cap:]
                        for w in move:
                            nop = mybir.InstNoOp(name=f"waitfix_{k}", text_hint="wait_fixup")
                            k += 1
                            nop.engine = inst.engine
                            nop.sync_info = type(si)(on_wait=[w], on_update=[])
                            out.append(nop)
                        inst.sync_info = type(si)(on_wait=list(keep), on_update=list(si.on_update))
                out.append(inst)
            blk.instructions = out
    return k


def _prep_edges(src_all, dst_all, per, nb, npad):
    """Tile edges per core by (dst block, src half), build gather idx arrays.

    Returns (blk_of_tile, start_t, stop_t, bufcol, n_lo, Tpad,
    srcidxs, dlocs, dlocTs): srcidx is the per-core [128, Tpad*8] int16 SBUF
    image; dloc is [P, Tpad] f32 dst-slot per edge slot (buffer order, -1
    pad); dlocT is [1, Tpad*128] bf16 dst-slot along free dim (buffer order).
    """
    NHALF = npad // 2
    percore = []
    cnt_bh = np.zeros((nb, 2), np.int64)
    for c in range(NCORES):
        m = (dst_all // per) == c
        s = src_all[m]
        loc = dst_all[m] - c * per
        b = loc // P
        hf = (s >= NHALF).astype(np.int64)
        order = np.lexsort((hf, b))
        s, loc, b, hf = s[order], loc[order], b[order], hf[order]
        cnt = np.bincount(b * 2 + hf, minlength=nb * 2).reshape(nb, 2)
        cnt_bh = np.maximum(cnt_bh, cnt)
        percore.append((s, loc, b, hf, cnt))

    tiles_bh = (cnt_bh + P - 1) // P            # [nb, 2]
    # processing-order tiles
    blk_l, hf_l = [], []
    tstart = np.zeros((nb, 2), np.int64)
    t = 0
    for b in range(nb):
        for hf in (0, 1):
            tstart[b, hf] = t
            n = int(tiles_bh[b, hf])
            blk_l += [b] * n
            hf_l += [hf] * n
            t += n
    T = t
    nchunks = (T + CT - 1) // CT
    Tpad = nchunks * CT
    blk_of_tile = np.array(blk_l + [nb - 1] * (Tpad - T), np.int64)
    hf_of_tile = np.array(hf_l + [0] * (Tpad - T), np.int64)
    start_t = np.zeros(Tpad, bool)
    stop_t = np.zeros(Tpad, bool)
    for b in range(nb):
        w = np.nonzero(blk_of_tile == b)[0]
        start_t[w[0]] = True
        stop_t[w[-1]] = True

    # buffer-column mapping: per chunk, lo tiles first then hi tiles
    bufcol = np.zeros(Tpad, np.int64)
    n_lo = []
    for ch in range(nchunks):
        ts = np.arange(ch * CT, (ch + 1) * CT)
        lo = ts[hf_of_tile[ts] == 0]
        hi = ts[hf_of_tile[ts] == 1]
        bufcol[lo] = np.arange(len(lo))
        bufcol[hi] = len(lo) + np.arange(len(hi))
        n_lo.append(len(lo))
    g2b = (np.arange(Tpad) // CT) * CT + bufcol   # proc tile -> buffer col

    import ml_dtypes
    SW = Tpad * 8
    srcidxs, dlocs, dlocTs = [], [], []
    for c in range(NCORES):
        s, loc, b, hf, cnt = percore[c]
        srcv = np.zeros((P, Tpad), np.int64)
        dlp = np.full((P, Tpad), -1.0, np.float32)   # proc order
        off = np.zeros(nb * 2 + 1, np.int64)
        off[1:] = np.cumsum(cnt.reshape(-1))
        key = b * 2 + hf
        pos = np.arange(len(s)) - off[key]
        tt = tstart[b, hf] + pos // P
        pp = pos % P
        srcv[pp, tt] = s - hf * NHALF
        dlp[pp, tt] = loc % P
        # sbuf idx image: slot (p, t) -> row 16k+p%16, col (bufgcol*8)+p//16
        p_g, t_g = np.mgrid[0:P, 0:Tpad]
        col = g2b[t_g] * 8 + p_g // 16
        row = p_g % 16
        si = np.zeros((P, SW), np.int16)
        for k in range(8):
            si[16 * k + row, col] = srcv
        srcidxs.append(si)
        dloc_buf = np.full((P, Tpad), -1.0, np.float32)
        dloc_buf[:, g2b] = dlp
        dlocs.append(dloc_buf)
        dT = np.full((Tpad, P), -1.0, np.float32)
        dT[g2b, :] = dlp.T
        dlocTs.append(dT.reshape(1, Tpad * P).astype(ml_dtypes.bfloat16))
    return (blk_of_tile, start_t, stop_t, bufcol, n_lo, Tpad,
            srcidxs, dlocs, dlocTs)


def _build(npad, Tpad, blk_of_tile, start_t, stop_t, bufcol, n_lo):
    per = npad // NCORES
    nb = per // P
    NHALF = npad // 2
    nlayers = 3
    nchunks = Tpad // CT
    f32 = mybir.dt.float32
    bf16 = mybir.dt.bfloat16
    i16 = mybir.dt.int16

    nc = bass.Bass(num_devices=NCORES, num_swdge_queues=NSWQ)
    # ---- dram I/O
    xT_d = nc.dram_tensor("xT", [P, per], bf16, kind="ExternalInput")
    waug_d = nc.dram_tensor("waug", [nlayers, P, AUGW], bf16, kind="ExternalInput")
    wh_d = nc.dram_tensor("wh", [P, OUT], f32, kind="ExternalInput")
    iota_d = nc.dram_tensor("iota", [P, 4 * P], f32, kind="ExternalInput")
    iotap_d = nc.dram_tensor("iotap", [P, 1], bf16, kind="ExternalInput")
    ident_d = nc.dram_tensor("ident", [P, P], f32, kind="ExternalInput")
    SW = Tpad * 8
    srcidx_d = nc.dram_tensor("srcidx", [P, SW], i16, kind="ExternalInput")
    dloc_d = nc.dram_tensor("dloc", [P, Tpad], f32, kind="ExternalInput")
    dlocT_d = nc.dram_tensor("dlocT", [1, Tpad * P], bf16, kind="ExternalInput")
    batchf_d = nc.dram_tensor("batchf", [P, nb], f32, kind="ExternalInput")
    out_d = nc.dram_tensor("out", [GRAPHS, OUT], f32, kind="ExternalOutput")

    h_loc = [nc.dram_tensor(f"h_loc{l}", [per, TW], bf16) for l in range(nlayers)]
    h_full = [nc.dram_tensor(f"h_full{l}", [npad, TW], bf16, addr_space="Shared")
              for l in range(nlayers)]

    groups = [list(range(NCORES))]

    with ExitStack() as ctx:
        tc = ctx.enter_context(tile.TileContext(nc))
        sb = ctx.enter_context(tc.tile_pool(name="sb", bufs=1))
        sb_g = ctx.enter_context(tc.tile_pool(name="sbg", bufs=2))
        sb_s = ctx.enter_context(tc.tile_pool(name="sbs", bufs=2))
        sb_t = ctx.enter_context(tc.tile_pool(name="sbt", bufs=2))
        sb_w = ctx.enter_context(tc.tile_pool(name="sbw", bufs=3))
        ps_h = ctx.enter_context(tc.tile_pool(name="psh", bufs=1, space="PSUM"))
        ps_agg = ctx.enter_context(tc.tile_pool(name="psagg", bufs=2, space="PSUM"))
        ps_ad = ctx.enter_context(tc.tile_pool(name="psad", bufs=2, space="PSUM"))
        ps_xp = ctx.enter_context(tc.tile_pool(name="psxp", bufs=1, space="PSUM"))
        ps_fin = ctx.enter_context(tc.tile_pool(name="psfin", bufs=1, space="PSUM"))

        # ---- persistent SBUF state
        xT = sb.tile([P, per], bf16)
        nc.sync.dma_start(out=xT[:], in_=xT_d[:])
        waug = sb.tile([P, nlayers, AUGW], bf16)
        nc.sync.dma_start(out=waug[:],
                          in_=waug_d[:].rearrange("l p a -> p l a"))
        wh = sb.tile([P, OUT], f32)
        nc.sync.dma_start(out=wh[:], in_=wh_d[:])
        iota = sb.tile([P, 4, P], f32)
        nc.sync.dma_start(out=iota[:].rearrange("p a b -> p (a b)"), in_=iota_d[:])
        iotap = sb.tile([P, 1], bf16)
        nc.sync.dma_start(out=iotap[:], in_=iotap_d[:])
        srci = sb.tile([P, SW], i16)
        nc.sync.dma_start(out=srci[:], in_=srcidx_d[:])
        dloc = sb.tile([P, Tpad, 1], f32)
        nc.sync.dma_start(out=dloc[:].rearrange("p t o -> p (t o)"), in_=dloc_d[:])
        batchf = sb.tile([P, nb, 1], f32)
        nc.sync.dma_start(out=batchf[:].rearrange("p b o -> p (b o)"), in_=batchf_d[:])
        identf = sb.tile([P, P], f32)
        nc.sync.dma_start(out=identf[:], in_=ident_d[:])

        nc.gpsimd.load_library(library_config.mlp)

        _regs = {}

        def nreg(v):
            if v not in _regs:
                _regs[v] = nc.gpsimd.to_reg(v)
            return _regs[v]

        hsb = sb.tile([P, nb, TBL], bf16)
        adsb = sb.tile([P, nb, H], bf16)
        pooled_ps = ps_fin.tile([GRAPHS, HC], f32)
        qn = [0]

        def nextq():
            qn[0] = (qn[0] + 1) % NSWQ
            return qn[0]

        for l in range(3):
            # ===== node phase =====
            for b in range(nb):
                ps = ps_h.tile([P, AUGW], f32, tag="ndps")
                nc.tensor.matmul(ps[:], lhsT=xT[:, b * P:(b + 1) * P],
                                 rhs=waug[:, l, :], start=True, stop=True)
                nc.vector.tensor_copy(out=hsb[:, b, :], in_=ps[:, :TBL])
                nc.vector.tensor_copy(out=adsb[:, b, :], in_=ps[:, TBL:AUGW])
            dh = nc.sync.dma_start(
                out=h_loc[l][:, 0:TBL].rearrange("(b p) d -> p b d", p=P),
                in_=hsb[:])
            cch = nc.gpsimd.collective_compute(
                "AllGather", mybir.AluOpType.bypass, replica_groups=groups,
                ins=[h_loc[l][:]], outs=[h_full[l][:]])
            add_dep_helper(cch.ins, dh.ins, sync=True, reason="h write before ag")

            # ===== edge phase =====
            agg_of_blk = {}
            for ci in range(nchunks):
                t0 = ci * CT
                nlo = int(n_lo[ci])
                g = sb_g.tile([P, CT, TW], bf16, tag="gath")
                if nlo > 0:
                    glo = nc.gpsimd.dma_gather(
                        out_ap=g[:, 0:nlo, :], in_ap=h_full[l][0:NHALF, :],
                        idxs_ap=srci[:, t0 * 8:t0 * 8 + nlo * 8],
                        num_idxs=nlo * P, num_idxs_reg=nreg(nlo * P), elem_size=TW,
                        single_packet=False, queue_num=nextq())
                    add_dep_helper(glo.ins, cch.ins, sync=True, reason="gather after ag")
                if nlo < CT:
                    ghi = nc.gpsimd.dma_gather(
                        out_ap=g[:, nlo:CT, :], in_ap=h_full[l][NHALF:npad, :],
                        idxs_ap=srci[:, t0 * 8 + nlo * 8:(t0 + CT) * 8],
                        num_idxs=(CT - nlo) * P, num_idxs_reg=nreg((CT - nlo) * P),
                        elem_size=TW, single_packet=False, queue_num=nextq())
                    add_dep_helper(ghi.ins, cch.ins, sync=True, reason="gather after ag")

                # transposed selectors (node-slot one-hot along partitions)
                dT = sb_t.tile([P, CT, P], bf16, tag="dT")
                nc.sync.dma_start(
                    out=dT[:].rearrange("p c e -> p (c e)"),
                    in_=dlocT_d[0:1, t0 * P:(t0 + CT) * P].to_broadcast(
                        [P, CT * P]))
                selT = sb_t.tile([P, CT, P], bf16, tag="selT")
                nc.vector.tensor_tensor(
                    out=selT[:], in0=dT[:],
                    in1=iotap[:].rearrange("p (c e) -> p c e", c=1).to_broadcast(
                        [P, CT, P]),
                    op=mybir.AluOpType.is_equal)
                # per-edge alpha_dst via PE: adps[:, j, :] = selT_j^T @ adsb[b]
                adps = ps_ad.tile([P, CT, H], f32, tag="adps")
                for t in range(t0, t0 + CT):
                    b = int(blk_of_tile[t])
                    j = int(bufcol[t])
                    nc.tensor.matmul(adps[:, j, :], lhsT=selT[:, j, :],
                                     rhs=adsb[:, b, :], start=True, stop=True)

                # e = lrelu(a_s + a_d); w = exp(e)
                lg = sb_w.tile([P, CT, H], f32, tag="lg")
                nc.vector.tensor_tensor(out=lg[:], in0=adps[:],
                                        in1=g[:, :, HC:TBL],
                                        op=mybir.AluOpType.add)
                lr = sb_w.tile([P, CT, H], f32, tag="lr")
                nc.vector.tensor_scalar_mul(lr[:], lg[:], NEG_SLOPE)
                nc.vector.tensor_tensor(out=lr[:], in0=lr[:], in1=lg[:],
                                        op=mybir.AluOpType.max)
                ex = sb_w.tile([P, CT, H], f32, tag="ex")
                nc.scalar.activation(ex[:], lr[:],
                                     mybir.ActivationFunctionType.Exp)
                # msg in place: g[:, :, h*Ch:(h+1)*Ch] *= ex[h]; cols HC:TBL = ex
                nc.vector.tensor_tensor(
                    out=g[:, :, 0:HC].rearrange("p c (h w) -> p c h w", h=H),
                    in0=g[:, :, 0:HC].rearrange("p c (h w) -> p c h w", h=H),
                    in1=ex[:].rearrange("p c (h o) -> p c h o", o=1).to_broadcast(
                        [P, CT, H, Ch]),
                    op=mybir.AluOpType.mult)
                nc.vector.tensor_copy(out=g[:, :, HC:TBL], in_=ex[:])
                # selectors for the whole chunk (dst one-hot per edge)
                sel = sb_s.tile([P, CT, P], bf16, tag="sel")
                for q in range(CT // 4):
                    nc.vector.tensor_tensor(
                        out=sel[:, 4 * q:4 * q + 4, :],
                        in0=dloc[:, t0 + 4 * q:t0 + 4 * q + 4, :].to_broadcast(
                            [P, 4, P]),
                        in1=iota[:], op=mybir.AluOpType.is_equal)
                # aggregate per tile (processing order)
                for t in range(t0, t0 + CT):
                    b = int(blk_of_tile[t])
                    j = int(bufcol[t])
                    if start_t[t]:
                        agg_of_blk[b] = ps_agg.tile([P, TBL], f32, tag="agg",
                                                    name=f"agg{l}_{b}")
                    nc.tensor.matmul(agg_of_blk[b][:], lhsT=sel[:, j, :],
                                     rhs=g[:, j, 0:TBL],
                                     start=bool(start_t[t]),
                                     stop=bool(stop_t[t]))
                    if stop_t[t]:
                        agg = agg_of_blk.pop(b)
                        rec = sb_w.tile([P, H], f32, tag="rec")
                        nc.vector.reciprocal(rec[:], agg[:, HC:TBL])
                        xb = sb_w.tile([P, HC], f32, tag="xb")
                        nc.vector.tensor_tensor(
                            out=xb[:].rearrange("p (h w) -> p h w", h=H),
                            in0=agg[:, 0:HC].rearrange("p (h w) -> p h w", h=H),
                            in1=rec[:].rearrange("p (h o) -> p h o", o=1)
                                .to_broadcast([P, H, Ch]),
                            op=mybir.AluOpType.mult)
                        nc.vector.tensor_scalar_max(xb[:], xb[:], 0.0)
                        if l < 2:
                            xps = ps_xp.tile([P, P], f32, tag="xps")
                            nc.tensor.transpose(xps[:], xb[:], identf[:])
                            nc.vector.tensor_copy(
                                out=xT[:, b * P:(b + 1) * P], in_=xps[:])
                        else:
                            bsel = sb_w.tile([P, GRAPHS], f32, tag="bsel")
                            nc.vector.tensor_tensor(
                                out=bsel[:],
                                in0=batchf[:, b, :].to_broadcast([P, GRAPHS]),
                                in1=iota[:, 0, :GRAPHS],
                                op=mybir.AluOpType.is_equal)
                            nc.tensor.matmul(pooled_ps[:], lhsT=bsel[:],
                                             rhs=xb[:], start=(b == 0),
                                             stop=(b == nb - 1))

        # ===== head =====
        pooled_sb = sb.tile([GRAPHS, HC], f32)
        nc.vector.tensor_copy(out=pooled_sb[:], in_=pooled_ps[:])
        pT_ps = ps_xp.tile([P, GRAPHS], f32, tag="xps")
        nc.tensor.transpose(pT_ps[:], pooled_sb[:], identf[:GRAPHS, :GRAPHS])
        pT_sb = sb.tile([P, GRAPHS], f32)
        nc.vector.tensor_copy(out=pT_sb[:], in_=pT_ps[:])
        log_ps = ps_xp.tile([GRAPHS, OUT], f32, tag="logps")
        nc.tensor.matmul(log_ps[:], lhsT=pT_sb[:], rhs=wh[:], start=True, stop=True)
        log_sb = sb.tile([GRAPHS, OUT], f32)
        nc.vector.tensor_copy(out=log_sb[:], in_=log_ps[:])
        nc.sync.dma_start(out=out_d[:], in_=log_sb[:])

    _fixup_wait_limits(nc)
    mybir.codegen_inst_isa_subclasses(nc)
    return nc


def prepare(x, Ws, a_srcs, a_dsts, biases, Wh, bh, edge_index, batch):
    n = x.shape[0]
    npad = int(math.ceil(n / (NCORES * P)) * NCORES * P)
    per = npad // NCORES
    nb = per // P

    x = np.asarray(x, np.float32)
    Ws = [np.asarray(w, np.float32) for w in Ws]
    a_srcs = [np.asarray(a, np.float32) for a in a_srcs]
    a_dsts = [np.asarray(a, np.float32) for a in a_dsts]
    Wh = np.asarray(Wh, np.float32)
    bh = np.asarray(bh, np.float32)
    edge_index = np.asarray(edge_index)
    batch = np.asarray(batch)
    for b in biases:
        assert np.allclose(np.asarray(b), 0.0), "nonzero GAT biases unsupported"

    import ml_dtypes
    # W_aug = [W | W@As | W@Ad]
    waugs = []
    for l in range(3):
        As = np.zeros((HC, H), np.float32)
        Ad = np.zeros((HC, H), np.float32)
        for h in range(H):
            As[h * Ch:(h + 1) * Ch, h] = a_srcs[l][h]
            Ad[h * Ch:(h + 1) * Ch, h] = a_dsts[l][h]
        W = Ws[l]
        waugs.append(np.concatenate([W, W @ As, W @ Ad], axis=1))
    waug = np.stack(waugs, 0).astype(ml_dtypes.bfloat16)  # [3, 128, AUGW]

    # edges + self loops (incl. pad nodes, so every row has >=1 edge)
    src_all = np.concatenate([edge_index[0].astype(np.int64),
                              np.arange(npad, dtype=np.int64)])
    dst_all = np.concatenate([edge_index[1].astype(np.int64),
                              np.arange(npad, dtype=np.int64)])
    (blk_of_tile, start_t, stop_t, bufcol, n_lo, Tpad,
     srcidxs, dlocs, dlocTs) = _prep_edges(src_all, dst_all, per, nb, npad)

    xpad = np.zeros((npad, HC), np.float32)
    xpad[:n] = x
    iota = np.tile(np.arange(P, dtype=np.float32)[None, :], (P, 4))

    batchf_full = np.full(npad, -1.0, np.float32)
    batchf_full[:n] = batch.astype(np.float32)

    nc = _build(npad, Tpad, blk_of_tile, start_t, stop_t, bufcol, n_lo)

    in_maps = []
    for c in range(NCORES):
        sl = slice(c * per, (c + 1) * per)
        in_maps.append({
            "xT": np.ascontiguousarray(xpad[sl].T).astype(ml_dtypes.bfloat16),
            "waug": waug,
            "wh": Wh,
            "iota": iota,
            "iotap": np.arange(P, dtype=np.float32)[:, None].astype(
                ml_dtypes.bfloat16),
            "ident": np.eye(P, dtype=np.float32),
            "srcidx": srcidxs[c],
            "dloc": dlocs[c],
            "dlocT": dlocTs[c],
            "batchf": np.ascontiguousarray(
                batchf_full[sl].reshape(nb, P).T),
            })
    return nc, in_maps


def run_gat(x, Ws, a_srcs, a_dsts, biases, Wh, bh, edge_index, batch):
    nc, in_maps = prepare(x, Ws, a_srcs, a_dsts, biases, Wh, bh,
                          edge_index, batch)
    res = run_bass_kernel_spmd(nc, in_maps, list(range(NCORES)))
    global LAST_EXEC_NS
    LAST_EXEC_NS = getattr(res, "exec_time_ns", None)
    logits = np.zeros((GRAPHS, OUT), np.float32)
    for c in range(NCORES):
        logits += res.results[c]["out"]
    return logits + bh


def kernel(**inputs):
    return np.asarray(run_gat(
        inputs["x"], inputs["Ws"], inputs["a_srcs"], inputs["a_dsts"],
        inputs["biases"], inputs["Wh"], inputs["bh"], inputs["edge_index"],
        inputs["batch"]), np.float32)


# revision 18
# speedup vs baseline: 2.1962x; 1.1255x over previous
"""Trainium2 Bass kernel for 3-layer GAT + global_add_pool + linear head.

Design (v3):
- Nodes (and incoming edges) sharded across 8 cores by dst.
- Node phase per layer: h_aug = x @ [W | W@As | W@Ad] on PE (bf16 in, f32 out).
  [h|alpha_src] rows (bf16, 512B stride) go to a DRAM table that is
  AllGathered across cores.
- Edge phase: edges tiled 128/dst-block (sub-split by src half for int16
  gather indices); per chunk of CT tiles, batched InstDMAGatherAnt fetches
  h[src] rows on 4 SWDGE queues. Per-edge alpha_dst comes from transposed
  selector matmuls on the (otherwise idle) PE; selectors are built by vector
  is_equal against a streamed dst-slot pattern.
- e = lrelu(a_s+a_d) on vector (max(z,.2z)), exp batched on scalar, messages
  scaled in place (bf16), then per-tile selector matmuls accumulate [w*h | w]
  into PSUM per dst block; normalization + relu on block end.
- Partial pooled logits summed on host.

Self-contained: no file reads; shapes hardcoded via constants.
"""
import math
import numpy as np
from contextlib import ExitStack

import concourse.bass as bass
import concourse.mybir as mybir
import concourse.tile as tile
from concourse.bass_utils import run_bass_kernel_spmd
from concourse.tile_rust import add_dep_helper
from concourse import library_config

NCORES = 8
P = 128
H = 4
Ch = 32
HC = 128
AUGW = HC + 2 * H   # 136: node matmul out [h | a_src | a_dst]
TBL = HC + H        # 132: useful row [h | a_src]
TW = 256            # bf16 table row width (512B stride)
NEG_SLOPE = 0.2
GRAPHS = 64
OUT = 10
CT = 32             # tiles (of 128 edges) per chunk
NSWQ = 4            # SWDGE queues
import os as _os
K_SCAL = _os.environ.get("K_SCAL", "1") == "1"

# instruction types whose BIR struct cannot carry all Tile-emitted waits
_WAIT_CAPS = {
    "InstDMAGatherAnt": 0,
    "InstDMAScatterAddAnt": 0,
    "InstNoOp": 1,
    "InstDrain": 1,
    "InstCollectiveCompute": 1,
}


def _fixup_wait_limits(nc):
    k = 0
    for fn in nc.m.functions:
        for blk in fn.blocks:
            out = []
            for inst in blk.instructions:
                cap = _WAIT_CAPS.get(type(inst).__name__, 1)
                si = inst.sync_info
                if si is not None:
                    waits = list(si.on_wait)
                    if len(waits) > cap:
                        keep, move = waits[:cap], waits[cap:]
                        for w in move:
                            nop = mybir.InstNoOp(name=f"waitfix_{k}", text_hint="wait_fixup")
                            k += 1
                            nop.engine = inst.engine
                            nop.sync_info = type(si)(on_wait=[w], on_update=[])
                            out.append(nop)
                        inst.sync_info = type(si)(on_wait=list(keep), on_update=list(si.on_update))
                out.append(inst)
            blk.instructions = out
    return k


def _prep_edges(src_all, dst_all, per, nb, npad):
    """Tile edges per core by (dst block, src half), build gather idx arrays.

    Returns (blk_of_tile, start_t, stop_t, bufcol, n_lo, Tpad,
    srcidxs, dlocs, dlocTs): srcidx is the per-core [128, Tpad*8] int16 SBUF
    image; dloc is [P, Tpad] f32 dst-slot per edge slot (buffer order, -1
    pad); dlocT is [1, Tpad*128] bf16 dst-slot along free dim (buffer order).
    """
    NHALF = npad // 2
    percore = []
    cnt_bh = np.zeros((nb, 2), np.int64)
    for c in range(NCORES):
        m = (dst_all // per) == c
        s = src_all[m]
        loc = dst_all[m] - c * per
        b = loc // P
        hf = (s >= NHALF).astype(np.int64)
        order = np.lexsort((hf, b))
        s, loc, b, hf = s[order], loc[order], b[order], hf[order]
        cnt = np.bincount(b * 2 + hf, minlength=nb * 2).reshape(nb, 2)
        cnt_bh = np.maximum(cnt_bh, cnt)
        percore.append((s, loc, b, hf, cnt))

    tiles_bh = (cnt_bh + P - 1) // P            # [nb, 2]
    # processing-order tiles
    blk_l, hf_l = [], []
    tstart = np.zeros((nb, 2), np.int64)
    t = 0
    for b in range(nb):
        for hf in (0, 1):
            tstart[b, hf] = t
            n = int(tiles_bh[b, hf])
            blk_l += [b] * n
            hf_l += [hf] * n
            t += n
    T = t
    nchunks = (T + CT - 1) // CT
    Tpad = nchunks * CT
    blk_of_tile = np.array(blk_l + [nb - 1] * (Tpad - T), np.int64)
    hf_of_tile = np.array(hf_l + [0] * (Tpad - T), np.int64)
    start_t = np.zeros(Tpad, bool)
    stop_t = np.zeros(Tpad, bool)
    for b in range(nb):
        w = np.nonzero(blk_of_tile == b)[0]
        start_t[w[0]] = True
        stop_t[w[-1]] = True

    # buffer-column mapping: per chunk, lo tiles first then hi tiles
    bufcol = np.zeros(Tpad, np.int64)
    n_lo = []
    for ch in range(nchunks):
        ts = np.arange(ch * CT, (ch + 1) * CT)
        lo = ts[hf_of_tile[ts] == 0]
        hi = ts[hf_of_tile[ts] == 1]
        bufcol[lo] = np.arange(len(lo))
        bufcol[hi] = len(lo) + np.arange(len(hi))
        n_lo.append(len(lo))
    g2b = (np.arange(Tpad) // CT) * CT + bufcol   # proc tile -> buffer col

    import ml_dtypes
    SW = Tpad * 8
    srcidxs, dlocs, dlocTs = [], [], []
    for c in range(NCORES):
        s, loc, b, hf, cnt = percore[c]
        srcv = np.zeros((P, Tpad), np.int64)
        dlp = np.full((P, Tpad), -1.0, np.float32)   # proc order
        off = np.zeros(nb * 2 + 1, np.int64)
        off[1:] = np.cumsum(cnt.reshape(-1))
        key = b * 2 + hf
        pos = np.arange(len(s)) - off[key]
        tt = tstart[b, hf] + pos // P
        pp = pos % P
        srcv[pp, tt] = s - hf * NHALF
        dlp[pp, tt] = loc % P
        # sbuf idx image: slot (p, t) -> row 16k+p%16, col (bufgcol*8)+p//16
        p_g, t_g = np.mgrid[0:P, 0:Tpad]
        col = g2b[t_g] * 8 + p_g // 16
        row = p_g % 16
        si = np.zeros((P, SW), np.int16)
        for k in range(8):
            si[16 * k + row, col] = srcv
        srcidxs.append(si)
        dloc_buf = np.full((P, Tpad), -1.0, np.float32)
        dloc_buf[:, g2b] = dlp
        dlocs.append(dloc_buf.astype(ml_dtypes.bfloat16))
        dT = np.full((Tpad, P), -1.0, np.float32)
        dT[g2b, :] = dlp.T
        dlocTs.append(dT.reshape(1, Tpad * P).astype(ml_dtypes.bfloat16))
    return (blk_of_tile, start_t, stop_t, bufcol, n_lo, Tpad,
            srcidxs, dlocs, dlocTs)


def _build(npad, Tpad, blk_of_tile, start_t, stop_t, bufcol, n_lo):
    per = npad // NCORES
    nb = per // P
    NHALF = npad // 2
    nlayers = 3
    nchunks = Tpad // CT
    f32 = mybir.dt.float32
    bf16 = mybir.dt.bfloat16
    i16 = mybir.dt.int16

    nc = bass.Bass(num_devices=NCORES, num_swdge_queues=NSWQ)
    # ---- dram I/O
    xT_d = nc.dram_tensor("xT", [P, per], bf16, kind="ExternalInput")
    waug_d = nc.dram_tensor("waug", [nlayers, P, AUGW], bf16, kind="ExternalInput")
    wh_d = nc.dram_tensor("wh", [P, OUT], f32, kind="ExternalInput")
    iota_d = nc.dram_tensor("iota", [P, 4 * P], f32, kind="ExternalInput")
    iotap_d = nc.dram_tensor("iotap", [P, 1], bf16, kind="ExternalInput")
    ident_d = nc.dram_tensor("ident", [P, P], f32, kind="ExternalInput")
    SW = Tpad * 8
    srcidx_d = nc.dram_tensor("srcidx", [P, SW], i16, kind="ExternalInput")
    dloc_d = nc.dram_tensor("dloc", [P, Tpad], bf16, kind="ExternalInput")
    iotarep_d = nc.dram_tensor("iotarep", [P, CT * P], bf16, kind="ExternalInput")
    dlocT_d = nc.dram_tensor("dlocT", [1, Tpad * P], bf16, kind="ExternalInput")
    batchf_d = nc.dram_tensor("batchf", [P, nb], f32, kind="ExternalInput")
    out_d = nc.dram_tensor("out", [GRAPHS, OUT], f32, kind="ExternalOutput")

    h_loc = [nc.dram_tensor(f"h_loc{l}", [per, TW], bf16) for l in range(nlayers)]
    h_full = [nc.dram_tensor(f"h_full{l}", [npad, TW], bf16, addr_space="Shared")
              for l in range(nlayers)]

    groups = [list(range(NCORES))]

    with ExitStack() as ctx:
        tc = ctx.enter_context(tile.TileContext(nc))
        sb = ctx.enter_context(tc.tile_pool(name="sb", bufs=1))
        sb_g = ctx.enter_context(tc.tile_pool(name="sbg", bufs=3))
        sb_s = ctx.enter_context(tc.tile_pool(name="sbs", bufs=2))
        sb_t = ctx.enter_context(tc.tile_pool(name="sbt", bufs=2))
        sb_w = ctx.enter_context(tc.tile_pool(name="sbw", bufs=3))
        ps_h = ctx.enter_context(tc.tile_pool(name="psh", bufs=1, space="PSUM"))
        ps_agg = ctx.enter_context(tc.tile_pool(name="psagg", bufs=2, space="PSUM"))
        ps_ad = ctx.enter_context(tc.tile_pool(name="psad", bufs=2, space="PSUM"))
        ps_xp = ctx.enter_context(tc.tile_pool(name="psxp", bufs=1, space="PSUM"))
        ps_fin = ctx.enter_context(tc.tile_pool(name="psfin", bufs=1, space="PSUM"))

        # ---- persistent SBUF state
        xT = sb.tile([P, per], bf16)
        nc.sync.dma_start(out=xT[:], in_=xT_d[:])
        waug = sb.tile([P, nlayers, AUGW], bf16)
        nc.sync.dma_start(out=waug[:],
                          in_=waug_d[:].rearrange("l p a -> p l a"))
        wh = sb.tile([P, OUT], f32)
        nc.sync.dma_start(out=wh[:], in_=wh_d[:])
        iota = sb.tile([P, 4, P], f32)
        nc.sync.dma_start(out=iota[:].rearrange("p a b -> p (a b)"), in_=iota_d[:])
        iotap = sb.tile([P, 1], bf16)
        nc.sync.dma_start(out=iotap[:], in_=iotap_d[:])
        srci = sb.tile([P, SW], i16)
        nc.sync.dma_start(out=srci[:], in_=srcidx_d[:])
        dloc = sb.tile([P, Tpad, 1], bf16)
        nc.sync.dma_start(out=dloc[:].rearrange("p t o -> p (t o)"), in_=dloc_d[:])
        iotarep = sb.tile([P, CT, P], bf16)
        nc.sync.dma_start(out=iotarep[:].rearrange("p c e -> p (c e)"),
                          in_=iotarep_d[:])
        batchf = sb.tile([P, nb, 1], f32)
        nc.sync.dma_start(out=batchf[:].rearrange("p b o -> p (b o)"), in_=batchf_d[:])
        identf = sb.tile([P, P], f32)
        nc.sync.dma_start(out=identf[:], in_=ident_d[:])

        nc.gpsimd.load_library(library_config.mlp)

        _regs = {}

        def nreg(v):
            if v not in _regs:
                _regs[v] = nc.gpsimd.to_reg(v)
            return _regs[v]

        hsb = sb.tile([P, nb, TBL], bf16)
        adsb = sb.tile([P, nb, H], bf16)
        pooled_ps = ps_fin.tile([GRAPHS, HC], f32)
        qn = [0]

        def nextq():
            qn[0] = (qn[0] + 1) % NSWQ
            return qn[0]

        for l in range(3):
            # ===== node phase =====
            for b in range(nb):
                ps = ps_h.tile([P, AUGW], f32, tag="ndps")
                nc.tensor.matmul(ps[:], lhsT=xT[:, b * P:(b + 1) * P],
                                 rhs=waug[:, l, :], start=True, stop=True)
                nc.vector.tensor_copy(out=hsb[:, b, :], in_=ps[:, :TBL])
                nc.vector.tensor_copy(out=adsb[:, b, :], in_=ps[:, TBL:AUGW])
            dh = nc.sync.dma_start(
                out=h_loc[l][:, 0:TBL].rearrange("(b p) d -> p b d", p=P),
                in_=hsb[:])
            cch = nc.gpsimd.collective_compute(
                "AllGather", mybir.AluOpType.bypass, replica_groups=groups,
                ins=[h_loc[l][:]], outs=[h_full[l][:]])
            add_dep_helper(cch.ins, dh.ins, sync=True, reason="h write before ag")

            # ===== edge phase =====
            agg_of_blk = {}
            for ci in range(nchunks):
                t0 = ci * CT
                nlo = int(n_lo[ci])
                g = sb_g.tile([P, CT, TW], bf16, tag="gath")
                if nlo > 0:
                    glo = nc.gpsimd.dma_gather(
                        out_ap=g[:, 0:nlo, :], in_ap=h_full[l][0:NHALF, :],
                        idxs_ap=srci[:, t0 * 8:t0 * 8 + nlo * 8],
                        num_idxs=nlo * P, num_idxs_reg=nreg(nlo * P), elem_size=TW,
                        single_packet=False, queue_num=nextq())
                    add_dep_helper(glo.ins, cch.ins, sync=True, reason="gather after ag")
                if nlo < CT:
                    ghi = nc.gpsimd.dma_gather(
                        out_ap=g[:, nlo:CT, :], in_ap=h_full[l][NHALF:npad, :],
                        idxs_ap=srci[:, t0 * 8 + nlo * 8:(t0 + CT) * 8],
                        num_idxs=(CT - nlo) * P, num_idxs_reg=nreg((CT - nlo) * P),
                        elem_size=TW, single_packet=False, queue_num=nextq())
                    add_dep_helper(ghi.ins, cch.ins, sync=True, reason="gather after ag")

                # transposed selectors (node-slot one-hot along partitions)
                dT = sb_t.tile([P, CT, P], bf16, tag="dT")
                nc.sync.dma_start(
                    out=dT[:].rearrange("p c e -> p (c e)"),
                    in_=dlocT_d[0:1, t0 * P:(t0 + CT) * P].to_broadcast(
                        [P, CT * P]))
                selT = sb_t.tile([P, CT, P], bf16, tag="selT")
                nc.vector.tensor_tensor(
                    out=selT[:], in0=dT[:],
                    in1=iotap[:].rearrange("p (c e) -> p c e", c=1).to_broadcast(
                        [P, CT, P]),
                    op=mybir.AluOpType.is_equal)
                # per-edge alpha_dst via PE: adps[:, j, :] = selT_j^T @ adsb[b]
                adps = ps_ad.tile([P, CT, H], f32, tag="adps")
                for t in range(t0, t0 + CT):
                    b = int(blk_of_tile[t])
                    j = int(bufcol[t])
                    nc.tensor.matmul(adps[:, j, :], lhsT=selT[:, j, :],
                                     rhs=adsb[:, b, :], start=True, stop=True)

                # e = lrelu(a_s + a_d); w = exp(e) written into g cols HC:TBL
                lg = sb_w.tile([P, CT, H], f32, tag="lg")
                nc.vector.tensor_tensor(out=lg[:], in0=adps[:],
                                        in1=g[:, :, HC:TBL],
                                        op=mybir.AluOpType.add)
                lr = sb_w.tile([P, CT, H], f32, tag="lr")
                if K_SCAL:
                    nc.scalar.activation(lr[:], lg[:],
                                         mybir.ActivationFunctionType.Lrelu,
                                         alpha=NEG_SLOPE)
                else:
                    nc.vector.tensor_scalar_mul(lr[:], lg[:], NEG_SLOPE)
                    nc.vector.tensor_tensor(out=lr[:], in0=lr[:], in1=lg[:],
                                            op=mybir.AluOpType.max)
                nc.scalar.activation(g[:, :, HC:TBL], lr[:],
                                     mybir.ActivationFunctionType.Exp)
                # msg in place: g[:, :, h*Ch:(h+1)*Ch] *= w[h]
                nc.vector.tensor_tensor(
                    out=g[:, :, 0:HC].rearrange("p c (h w) -> p c h w", h=H),
                    in0=g[:, :, 0:HC].rearrange("p c (h w) -> p c h w", h=H),
                    in1=g[:, :, HC:TBL].rearrange("p c (h o) -> p c h o", o=1)
                        .to_broadcast([P, CT, H, Ch]),
                    op=mybir.AluOpType.mult)
                # selectors for the whole chunk (dst one-hot per edge)
                sel = sb_s.tile([P, CT, P], bf16, tag="sel")
                nc.vector.tensor_tensor(
                    out=sel[:], in0=iotarep[:],
                    in1=dloc[:, t0:t0 + CT, :].to_broadcast([P, CT, P]),
                    op=mybir.AluOpType.is_equal)
                # aggregate per tile (processing order)
                for t in range(t0, t0 + CT):
                    b = int(blk_of_tile[t])
                    j = int(bufcol[t])
                    if start_t[t]:
                        agg_of_blk[b] = ps_agg.tile([P, TBL], f32, tag="agg",
                                                    name=f"agg{l}_{b}")
                    nc.tensor.matmul(agg_of_blk[b][:], lhsT=sel[:, j, :],
                                     rhs=g[:, j, 0:TBL],
                                     start=bool(start_t[t]),
                                     stop=bool(stop_t[t]))
                    if stop_t[t]:
                        agg = agg_of_blk.pop(b)
                        rec = sb_w.tile([P, H], f32, tag="rec")
                        nc.vector.reciprocal(rec[:], agg[:, HC:TBL])
                        xb = sb_w.tile([P, HC], f32, tag="xb")
                        nc.vector.tensor_tensor(
                            out=xb[:].rearrange("p (h w) -> p h w", h=H),
                            in0=agg[:, 0:HC].rearrange("p (h w) -> p h w", h=H),
                            in1=rec[:].rearrange("p (h o) -> p h o", o=1)
                                .to_broadcast([P, H, Ch]),
                            op=mybir.AluOpType.mult)
                        if K_SCAL:
                            nc.scalar.activation(xb[:], xb[:],
                                                 mybir.ActivationFunctionType.Relu)
                        else:
                            nc.vector.tensor_scalar_max(xb[:], xb[:], 0.0)
                        if l < 2:
                            xps = ps_xp.tile([P, P], f32, tag="xps")
                            nc.tensor.transpose(xps[:], xb[:], identf[:])
                            nc.vector.tensor_copy(
                                out=xT[:, b * P:(b + 1) * P], in_=xps[:])
                        else:
                            bsel = sb_w.tile([P, GRAPHS], f32, tag="bsel")
                            nc.vector.tensor_tensor(
                                out=bsel[:],
                                in0=batchf[:, b, :].to_broadcast([P, GRAPHS]),
                                in1=iota[:, 0, :GRAPHS],
                                op=mybir.AluOpType.is_equal)
                            nc.tensor.matmul(pooled_ps[:], lhsT=bsel[:],
                                             rhs=xb[:], start=(b == 0),
                                             stop=(b == nb - 1))

        # ===== head =====
        pooled_sb = sb.tile([GRAPHS, HC], f32)
        nc.vector.tensor_copy(out=pooled_sb[:], in_=pooled_ps[:])
        pT_ps = ps_xp.tile([P, GRAPHS], f32, tag="xps")
        nc.tensor.transpose(pT_ps[:], pooled_sb[:], identf[:GRAPHS, :GRAPHS])
        pT_sb = sb.tile([P, GRAPHS], f32)
        nc.vector.tensor_copy(out=pT_sb[:], in_=pT_ps[:])
        log_ps = ps_xp.tile([GRAPHS, OUT], f32, tag="logps")
        nc.tensor.matmul(log_ps[:], lhsT=pT_sb[:], rhs=wh[:], start=True, stop=True)
        log_sb = sb.tile([GRAPHS, OUT], f32)
        nc.vector.tensor_copy(out=log_sb[:], in_=log_ps[:])
        nc.sync.dma_start(out=out_d[:], in_=log_sb[:])

    _fixup_wait_limits(nc)
    mybir.codegen_inst_isa_subclasses(nc)
    return nc


def prepare(x, Ws, a_srcs, a_dsts, biases, Wh, bh, edge_index, batch):
    n = x.shape[0]
    npad = int(math.ceil(n / (NCORES * P)) * NCORES * P)
    per = npad // NCORES
    nb = per // P

    x = np.asarray(x, np.float32)
    Ws = [np.asarray(w, np.float32) for w in Ws]
    a_srcs = [np.asarray(a, np.float32) for a in a_srcs]
    a_dsts = [np.asarray(a, np.float32) for a in a_dsts]
    Wh = np.asarray(Wh, np.float32)
    bh = np.asarray(bh, np.float32)
    edge_index = np.asarray(edge_index)
    batch = np.asarray(batch)
    for b in biases:
        assert np.allclose(np.asarray(b), 0.0), "nonzero GAT biases unsupported"

    import ml_dtypes
    # W_aug = [W | W@As | W@Ad]
    waugs = []
    for l in range(3):
        As = np.zeros((HC, H), np.float32)
        Ad = np.zeros((HC, H), np.float32)
        for h in range(H):
            As[h * Ch:(h + 1) * Ch, h] = a_srcs[l][h]
            Ad[h * Ch:(h + 1) * Ch, h] = a_dsts[l][h]
        W = Ws[l]
        waugs.append(np.concatenate([W, W @ As, W @ Ad], axis=1))
    waug = np.stack(waugs, 0).astype(ml_dtypes.bfloat16)  # [3, 128, AUGW]

    # edges + self loops (incl. pad nodes, so every row has >=1 edge)
    src_all = np.concatenate([edge_index[0].astype(np.int64),
                              np.arange(npad, dtype=np.int64)])
    dst_all = np.concatenate([edge_index[1].astype(np.int64),
                              np.arange(npad, dtype=np.int64)])
    (blk_of_tile, start_t, stop_t, bufcol, n_lo, Tpad,
     srcidxs, dlocs, dlocTs) = _prep_edges(src_all, dst_all, per, nb, npad)

    xpad = np.zeros((npad, HC), np.float32)
    xpad[:n] = x
    iota = np.tile(np.arange(P, dtype=np.float32)[None, :], (P, 4))

    batchf_full = np.full(npad, -1.0, np.float32)
    batchf_full[:n] = batch.astype(np.float32)

    nc = _build(npad, Tpad, blk_of_tile, start_t, stop_t, bufcol, n_lo)

    in_maps = []
    for c in range(NCORES):
        sl = slice(c * per, (c + 1) * per)
        in_maps.append({
            "xT": np.ascontiguousarray(xpad[sl].T).astype(ml_dtypes.bfloat16),
            "waug": waug,
            "wh": Wh,
            "iota": iota,
            "iotap": np.arange(P, dtype=np.float32)[:, None].astype(
                ml_dtypes.bfloat16),
            "ident": np.eye(P, dtype=np.float32),
            "srcidx": srcidxs[c],
            "dloc": dlocs[c],
            "iotarep": np.tile(np.arange(P, dtype=np.float32)[None, :],
                               (P, CT)).astype(ml_dtypes.bfloat16),
            "dlocT": dlocTs[c],
            "batchf": np.ascontiguousarray(
                batchf_full[sl].reshape(nb, P).T),
            })
    return nc, in_maps


def run_gat(x, Ws, a_srcs, a_dsts, biases, Wh, bh, edge_index, batch):
    nc, in_maps = prepare(x, Ws, a_srcs, a_dsts, biases, Wh, bh,
                          edge_index, batch)
    res = run_bass_kernel_spmd(nc, in_maps, list(range(NCORES)))
    global LAST_EXEC_NS
    LAST_EXEC_NS = getattr(res, "exec_time_ns", None)
    logits = np.zeros((GRAPHS, OUT), np.float32)
    for c in range(NCORES):
        logits += res.results[c]["out"]
    return logits + bh


def kernel(**inputs):
    return np.asarray(run_gat(
        inputs["x"], inputs["Ws"], inputs["a_srcs"], inputs["a_dsts"],
        inputs["biases"], inputs["Wh"], inputs["bh"], inputs["edge_index"],
        inputs["batch"]), np.float32)


# revision 21
# speedup vs baseline: 2.2261x; 1.0136x over previous
"""Trainium2 Bass kernel for 3-layer GAT + global_add_pool + linear head.

Design (v3):
- Nodes (and incoming edges) sharded across 8 cores by dst.
- Node phase per layer: h_aug = x @ [W | W@As | W@Ad] on PE (bf16 in, f32 out).
  [h|alpha_src] rows (bf16, 512B stride) go to a DRAM table that is
  AllGathered across cores.
- Edge phase: edges tiled 128/dst-block (sub-split by src half for int16
  gather indices); per chunk of CT tiles, batched InstDMAGatherAnt fetches
  h[src] rows on 4 SWDGE queues. Per-edge alpha_dst comes from transposed
  selector matmuls on the (otherwise idle) PE; selectors are built by vector
  is_equal against a streamed dst-slot pattern.
- e = lrelu(a_s+a_d) on vector (max(z,.2z)), exp batched on scalar, messages
  scaled in place (bf16), then per-tile selector matmuls accumulate [w*h | w]
  into PSUM per dst block; normalization + relu on block end.
- Partial pooled logits summed on host.

Self-contained: no file reads; shapes hardcoded via constants.
"""
import math
import numpy as np
from contextlib import ExitStack

import concourse.bass as bass
import concourse.mybir as mybir
import concourse.tile as tile
from concourse.bass_utils import run_bass_kernel_spmd
from concourse.tile_rust import add_dep_helper
from concourse import library_config

NCORES = 8
P = 128
H = 4
Ch = 32
HC = 128
AUGW = HC + 2 * H   # 136: node matmul out [h | a_src | a_dst]
TBL = HC + H        # 132: useful row [h | a_src]
TW = 256            # bf16 table row width (512B stride)
NEG_SLOPE = 0.2
GRAPHS = 64
OUT = 10
CT = 32             # tiles (of 128 edges) per chunk
NSWQ = 4            # SWDGE queues
import os as _os
K_LRELU = _os.environ.get("K_LRELU", "0") == "1"
K_RELU = _os.environ.get("K_RELU", "1") == "1"

# instruction types whose BIR struct cannot carry all Tile-emitted waits
_WAIT_CAPS = {
    "InstDMAGatherAnt": 0,
    "InstDMAScatterAddAnt": 0,
    "InstNoOp": 1,
    "InstDrain": 1,
    "InstCollectiveCompute": 1,
}


def _fixup_wait_limits(nc):
    k = 0
    for fn in nc.m.functions:
        for blk in fn.blocks:
            out = []
            for inst in blk.instructions:
                cap = _WAIT_CAPS.get(type(inst).__name__, 1)
                si = inst.sync_info
                if si is not None:
                    waits = list(si.on_wait)
                    if len(waits) > cap:
                        keep, move = waits[:cap], waits[cap:]
                        for w in move:
                            nop = mybir.InstNoOp(name=f"waitfix_{k}", text_hint="wait_fixup")
                            k += 1
                            nop.engine = inst.engine
                            nop.sync_info = type(si)(on_wait=[w], on_update=[])
                            out.append(nop)
                        inst.sync_info = type(si)(on_wait=list(keep), on_update=list(si.on_update))
                out.append(inst)
            blk.instructions = out
    return k


def _prep_edges(src_all, dst_all, per, nb, npad):
    """Tile edges per core by (dst block, src half), build gather idx arrays.

    Returns (blk_of_tile, start_t, stop_t, bufcol, n_lo, Tpad,
    srcidxs, dlocs, dlocTs): srcidx is the per-core [128, Tpad*8] int16 SBUF
    image; dloc is [P, Tpad] f32 dst-slot per edge slot (buffer order, -1
    pad); dlocT is [1, Tpad*128] bf16 dst-slot along free dim (buffer order).
    """
    NHALF = npad // 2
    percore = []
    cnt_bh = np.zeros((nb, 2), np.int64)
    for c in range(NCORES):
        m = (dst_all // per) == c
        s = src_all[m]
        loc = dst_all[m] - c * per
        b = loc // P
        hf = (s >= NHALF).astype(np.int64)
        order = np.lexsort((hf, b))
        s, loc, b, hf = s[order], loc[order], b[order], hf[order]
        cnt = np.bincount(b * 2 + hf, minlength=nb * 2).reshape(nb, 2)
        cnt_bh = np.maximum(cnt_bh, cnt)
        percore.append((s, loc, b, hf, cnt))

    tiles_bh = (cnt_bh + P - 1) // P            # [nb, 2]
    # processing-order tiles
    blk_l, hf_l = [], []
    tstart = np.zeros((nb, 2), np.int64)
    t = 0
    for b in range(nb):
        for hf in (0, 1):
            tstart[b, hf] = t
            n = int(tiles_bh[b, hf])
            blk_l += [b] * n
            hf_l += [hf] * n
            t += n
    T = t
    nchunks = (T + CT - 1) // CT
    Tpad = nchunks * CT
    blk_of_tile = np.array(blk_l + [nb - 1] * (Tpad - T), np.int64)
    hf_of_tile = np.array(hf_l + [0] * (Tpad - T), np.int64)
    start_t = np.zeros(Tpad, bool)
    stop_t = np.zeros(Tpad, bool)
    for b in range(nb):
        w = np.nonzero(blk_of_tile == b)[0]
        start_t[w[0]] = True
        stop_t[w[-1]] = True

    # buffer-column mapping: per chunk, lo tiles first then hi tiles
    bufcol = np.zeros(Tpad, np.int64)
    n_lo = []
    for ch in range(nchunks):
        ts = np.arange(ch * CT, (ch + 1) * CT)
        lo = ts[hf_of_tile[ts] == 0]
        hi = ts[hf_of_tile[ts] == 1]
        bufcol[lo] = np.arange(len(lo))
        bufcol[hi] = len(lo) + np.arange(len(hi))
        n_lo.append(len(lo))
    g2b = (np.arange(Tpad) // CT) * CT + bufcol   # proc tile -> buffer col

    import ml_dtypes
    SW = Tpad * 8
    srcidxs, dlocs, dlocTs = [], [], []
    for c in range(NCORES):
        s, loc, b, hf, cnt = percore[c]
        srcv = np.zeros((P, Tpad), np.int64)
        dlp = np.full((P, Tpad), -1.0, np.float32)   # proc order
        off = np.zeros(nb * 2 + 1, np.int64)
        off[1:] = np.cumsum(cnt.reshape(-1))
        key = b * 2 + hf
        pos = np.arange(len(s)) - off[key]
        tt = tstart[b, hf] + pos // P
        pp = pos % P
        srcv[pp, tt] = s - hf * NHALF
        dlp[pp, tt] = loc % P
        # sbuf idx image: slot (p, t) -> row 16k+p%16, col (bufgcol*8)+p//16
        p_g, t_g = np.mgrid[0:P, 0:Tpad]
        col = g2b[t_g] * 8 + p_g // 16
        row = p_g % 16
        si = np.zeros((P, SW), np.int16)
        for k in range(8):
            si[16 * k + row, col] = srcv
        srcidxs.append(si)
        dloc_buf = np.full((P, Tpad), -1.0, np.float32)
        dloc_buf[:, g2b] = dlp
        dlocs.append(dloc_buf.astype(ml_dtypes.bfloat16))
        dT = np.full((Tpad, P), -1.0, np.float32)
        dT[g2b, :] = dlp.T
        dlocTs.append(dT.reshape(1, Tpad * P).astype(ml_dtypes.bfloat16))
    return (blk_of_tile, start_t, stop_t, bufcol, n_lo, Tpad,
            srcidxs, dlocs, dlocTs)


def _build(npad, Tpad, blk_of_tile, start_t, stop_t, bufcol, n_lo):
    per = npad // NCORES
    nb = per // P
    NHALF = npad // 2
    nlayers = 3
    nchunks = Tpad // CT
    f32 = mybir.dt.float32
    bf16 = mybir.dt.bfloat16
    i16 = mybir.dt.int16

    nc = bass.Bass(num_devices=NCORES, num_swdge_queues=NSWQ)
    # ---- dram I/O
    xT_d = nc.dram_tensor("xT", [P, per], bf16, kind="ExternalInput")
    waug_d = nc.dram_tensor("waug", [nlayers, P, AUGW], bf16, kind="ExternalInput")
    wh_d = nc.dram_tensor("wh", [P, OUT], f32, kind="ExternalInput")
    iota_d = nc.dram_tensor("iota", [P, 4 * P], f32, kind="ExternalInput")
    iotap_d = nc.dram_tensor("iotap", [P, 1], bf16, kind="ExternalInput")
    ident_d = nc.dram_tensor("ident", [P, P], f32, kind="ExternalInput")
    SW = Tpad * 8
    srcidx_d = nc.dram_tensor("srcidx", [P, SW], i16, kind="ExternalInput")
    dloc_d = nc.dram_tensor("dloc", [P, Tpad], bf16, kind="ExternalInput")
    iotarep_d = nc.dram_tensor("iotarep", [P, CT * P], bf16, kind="ExternalInput")
    dlocT_d = nc.dram_tensor("dlocT", [1, Tpad * P], bf16, kind="ExternalInput")
    batchf_d = nc.dram_tensor("batchf", [P, nb], f32, kind="ExternalInput")
    out_d = nc.dram_tensor("out", [GRAPHS, OUT], f32, kind="ExternalOutput")

    h_loc = [nc.dram_tensor(f"h_loc{l}", [per, TW], bf16) for l in range(nlayers)]
    h_full = [nc.dram_tensor(f"h_full{l}", [npad, TW], bf16, addr_space="Shared")
              for l in range(nlayers)]

    groups = [list(range(NCORES))]

    with ExitStack() as ctx:
        tc = ctx.enter_context(tile.TileContext(nc))
        sb = ctx.enter_context(tc.tile_pool(name="sb", bufs=1))
        sb_g = ctx.enter_context(tc.tile_pool(name="sbg", bufs=4))
        sb_s = ctx.enter_context(tc.tile_pool(name="sbs", bufs=3))
        sb_t = ctx.enter_context(tc.tile_pool(name="sbt", bufs=3))
        sb_w = ctx.enter_context(tc.tile_pool(name="sbw", bufs=3))
        ps_h = ctx.enter_context(tc.tile_pool(name="psh", bufs=2, space="PSUM"))
        ps_agg = ctx.enter_context(tc.tile_pool(name="psagg", bufs=2, space="PSUM"))
        ps_ad = ctx.enter_context(tc.tile_pool(name="psad", bufs=2, space="PSUM"))
        ps_xp = ctx.enter_context(tc.tile_pool(name="psxp", bufs=1, space="PSUM"))
        ps_fin = ctx.enter_context(tc.tile_pool(name="psfin", bufs=1, space="PSUM"))

        # ---- persistent SBUF state
        xT = sb.tile([P, per], bf16)
        nc.sync.dma_start(out=xT[:], in_=xT_d[:])
        waug = sb.tile([P, nlayers, AUGW], bf16)
        nc.sync.dma_start(out=waug[:],
                          in_=waug_d[:].rearrange("l p a -> p l a"))
        wh = sb.tile([P, OUT], f32)
        nc.sync.dma_start(out=wh[:], in_=wh_d[:])
        iota = sb.tile([P, 4, P], f32)
        nc.sync.dma_start(out=iota[:].rearrange("p a b -> p (a b)"), in_=iota_d[:])
        iotap = sb.tile([P, 1], bf16)
        nc.sync.dma_start(out=iotap[:], in_=iotap_d[:])
        srci = sb.tile([P, SW], i16)
        nc.sync.dma_start(out=srci[:], in_=srcidx_d[:])
        dloc = sb.tile([P, Tpad, 1], bf16)
        nc.sync.dma_start(out=dloc[:].rearrange("p t o -> p (t o)"), in_=dloc_d[:])
        iotarep = sb.tile([P, CT, P], bf16)
        nc.sync.dma_start(out=iotarep[:].rearrange("p c e -> p (c e)"),
                          in_=iotarep_d[:])
        batchf = sb.tile([P, nb, 1], f32)
        nc.sync.dma_start(out=batchf[:].rearrange("p b o -> p (b o)"), in_=batchf_d[:])
        identf = sb.tile([P, P], f32)
        nc.sync.dma_start(out=identf[:], in_=ident_d[:])

        nc.gpsimd.load_library(library_config.mlp)

        _regs = {}

        def nreg(v):
            if v not in _regs:
                _regs[v] = nc.gpsimd.to_reg(v)
            return _regs[v]

        hsb = sb.tile([P, nb, TBL], bf16)
        adsb = sb.tile([P, nb, H], bf16)
        pooled_ps = ps_fin.tile([GRAPHS, HC], f32)
        qn = [0]

        def nextq():
            qn[0] = (qn[0] + 1) % NSWQ
            return qn[0]

        for l in range(3):
            # ===== node phase =====
            for b in range(nb):
                ps = ps_h.tile([P, AUGW], f32, tag="ndps")
                nc.tensor.matmul(ps[:], lhsT=xT[:, b * P:(b + 1) * P],
                                 rhs=waug[:, l, :], start=True, stop=True)
                nc.vector.tensor_copy(out=hsb[:, b, :], in_=ps[:, :TBL])
                nc.vector.tensor_copy(out=adsb[:, b, :], in_=ps[:, TBL:AUGW])
            dh = nc.sync.dma_start(
                out=h_loc[l][:, 0:TBL].rearrange("(b p) d -> p b d", p=P),
                in_=hsb[:])
            cch = nc.gpsimd.collective_compute(
                "AllGather", mybir.AluOpType.bypass, replica_groups=groups,
                ins=[h_loc[l][:]], outs=[h_full[l][:]])
            add_dep_helper(cch.ins, dh.ins, sync=True, reason="h write before ag")

            # ===== edge phase =====
            agg_of_blk = {}
            for ci in range(nchunks):
                t0 = ci * CT
                nlo = int(n_lo[ci])
                g = sb_g.tile([P, CT, TW], bf16, tag="gath")
                if nlo > 0:
                    glo = nc.gpsimd.dma_gather(
                        out_ap=g[:, 0:nlo, :], in_ap=h_full[l][0:NHALF, :],
                        idxs_ap=srci[:, t0 * 8:t0 * 8 + nlo * 8],
                        num_idxs=nlo * P, num_idxs_reg=nreg(nlo * P), elem_size=TW,
                        single_packet=False, queue_num=nextq())
                    add_dep_helper(glo.ins, cch.ins, sync=True, reason="gather after ag")
                if nlo < CT:
                    ghi = nc.gpsimd.dma_gather(
                        out_ap=g[:, nlo:CT, :], in_ap=h_full[l][NHALF:npad, :],
                        idxs_ap=srci[:, t0 * 8 + nlo * 8:(t0 + CT) * 8],
                        num_idxs=(CT - nlo) * P, num_idxs_reg=nreg((CT - nlo) * P),
                        elem_size=TW, single_packet=False, queue_num=nextq())
                    add_dep_helper(ghi.ins, cch.ins, sync=True, reason="gather after ag")

                # transposed selectors (node-slot one-hot along partitions)
                dT = sb_t.tile([P, CT, P], bf16, tag="dT")
                nc.sync.dma_start(
                    out=dT[:].rearrange("p c e -> p (c e)"),
                    in_=dlocT_d[0:1, t0 * P:(t0 + CT) * P].to_broadcast(
                        [P, CT * P]))
                selT = sb_t.tile([P, CT, P], bf16, tag="selT")
                nc.vector.tensor_tensor(
                    out=selT[:], in0=dT[:],
                    in1=iotap[:].rearrange("p (c e) -> p c e", c=1).to_broadcast(
                        [P, CT, P]),
                    op=mybir.AluOpType.is_equal)
                # per-edge alpha_dst via PE: adps[:, j, :] = selT_j^T @ adsb[b]
                adps = ps_ad.tile([P, CT, H], f32, tag="adps")
                for t in range(t0, t0 + CT):
                    b = int(blk_of_tile[t])
                    j = int(bufcol[t])
                    nc.tensor.matmul(adps[:, j, :], lhsT=selT[:, j, :],
                                     rhs=adsb[:, b, :], start=True, stop=True)

                # e = lrelu(a_s + a_d); w = exp(e) written into g cols HC:TBL
                lg = sb_w.tile([P, CT, H], f32, tag="lg")
                nc.vector.tensor_tensor(out=lg[:], in0=adps[:],
                                        in1=g[:, :, HC:TBL],
                                        op=mybir.AluOpType.add)
                lr = sb_w.tile([P, CT, H], f32, tag="lr")
                if K_LRELU:
                    nc.scalar.activation(lr[:], lg[:],
                                         mybir.ActivationFunctionType.Lrelu,
                                         alpha=NEG_SLOPE)
                else:
                    nc.vector.tensor_scalar_mul(lr[:], lg[:], NEG_SLOPE)
                    nc.vector.tensor_tensor(out=lr[:], in0=lr[:], in1=lg[:],
                                            op=mybir.AluOpType.max)
                nc.scalar.activation(g[:, :, HC:TBL], lr[:],
                                     mybir.ActivationFunctionType.Exp)
                # msg in place: g[:, :, h*Ch:(h+1)*Ch] *= w[h]
                nc.vector.tensor_tensor(
                    out=g[:, :, 0:HC].rearrange("p c (h w) -> p c h w", h=H),
                    in0=g[:, :, 0:HC].rearrange("p c (h w) -> p c h w", h=H),
                    in1=g[:, :, HC:TBL].rearrange("p c (h o) -> p c h o", o=1)
                        .to_broadcast([P, CT, H, Ch]),
                    op=mybir.AluOpType.mult)
                # selectors for the whole chunk (dst one-hot per edge)
                sel = sb_s.tile([P, CT, P], bf16, tag="sel")
                nc.vector.tensor_tensor(
                    out=sel[:], in0=iotarep[:],
                    in1=dloc[:, t0:t0 + CT, :].to_broadcast([P, CT, P]),
                    op=mybir.AluOpType.is_equal)
                # aggregate per tile (processing order)
                for t in range(t0, t0 + CT):
                    b = int(blk_of_tile[t])
                    j = int(bufcol[t])
                    if start_t[t]:
                        agg_of_blk[b] = ps_agg.tile([P, TBL], f32, tag="agg",
                                                    name=f"agg{l}_{b}")
                    nc.tensor.matmul(agg_of_blk[b][:], lhsT=sel[:, j, :],
                                     rhs=g[:, j, 0:TBL],
                                     start=bool(start_t[t]),
                                     stop=bool(stop_t[t]))
                    if stop_t[t]:
                        agg = agg_of_blk.pop(b)
                        rec = sb_w.tile([P, H], f32, tag="rec")
                        nc.vector.reciprocal(rec[:], agg[:, HC:TBL])
                        xb = sb_w.tile([P, HC], f32, tag="xb")
                        nc.vector.tensor_tensor(
                            out=xb[:].rearrange("p (h w) -> p h w", h=H),
                            in0=agg[:, 0:HC].rearrange("p (h w) -> p h w", h=H),
                            in1=rec[:].rearrange("p (h o) -> p h o", o=1)
                                .to_broadcast([P, H, Ch]),
                            op=mybir.AluOpType.mult)
                        if K_RELU:
                            nc.scalar.activation(xb[:], xb[:],
                                                 mybir.ActivationFunctionType.Relu)
                        else:
                            nc.vector.tensor_scalar_max(xb[:], xb[:], 0.0)
                        if l < 2:
                            xps = ps_xp.tile([P, P], f32, tag="xps")
                            nc.tensor.transpose(xps[:], xb[:], identf[:])
                            nc.vector.tensor_copy(
                                out=xT[:, b * P:(b + 1) * P], in_=xps[:])
                        else:
                            bsel = sb_w.tile([P, GRAPHS], f32, tag="bsel")
                            nc.vector.tensor_tensor(
                                out=bsel[:],
                                in0=batchf[:, b, :].to_broadcast([P, GRAPHS]),
                                in1=iota[:, 0, :GRAPHS],
                                op=mybir.AluOpType.is_equal)
                            nc.tensor.matmul(pooled_ps[:], lhsT=bsel[:],
                                             rhs=xb[:], start=(b == 0),
                                             stop=(b == nb - 1))

        # ===== head =====
        pooled_sb = sb.tile([GRAPHS, HC], f32)
        nc.vector.tensor_copy(out=pooled_sb[:], in_=pooled_ps[:])
        pT_ps = ps_xp.tile([P, GRAPHS], f32, tag="xps")
        nc.tensor.transpose(pT_ps[:], pooled_sb[:], identf[:GRAPHS, :GRAPHS])
        pT_sb = sb.tile([P, GRAPHS], f32)
        nc.vector.tensor_copy(out=pT_sb[:], in_=pT_ps[:])
        log_ps = ps_xp.tile([GRAPHS, OUT], f32, tag="xps")
        nc.tensor.matmul(log_ps[:], lhsT=pT_sb[:], rhs=wh[:], start=True, stop=True)
        log_sb = sb.tile([GRAPHS, OUT], f32)
        nc.vector.tensor_copy(out=log_sb[:], in_=log_ps[:])
        nc.sync.dma_start(out=out_d[:], in_=log_sb[:])

    _fixup_wait_limits(nc)
    mybir.codegen_inst_isa_subclasses(nc)
    return nc


def prepare(x, Ws, a_srcs, a_dsts, biases, Wh, bh, edge_index, batch):
    n = x.shape[0]
    npad = int(math.ceil(n / (NCORES * P)) * NCORES * P)
    per = npad // NCORES
    nb = per // P

    x = np.asarray(x, np.float32)
    Ws = [np.asarray(w, np.float32) for w in Ws]
    a_srcs = [np.asarray(a, np.float32) for a in a_srcs]
    a_dsts = [np.asarray(a, np.float32) for a in a_dsts]
    Wh = np.asarray(Wh, np.float32)
    bh = np.asarray(bh, np.float32)
    edge_index = np.asarray(edge_index)
    batch = np.asarray(batch)
    for b in biases:
        assert np.allclose(np.asarray(b), 0.0), "nonzero GAT biases unsupported"

    import ml_dtypes
    # W_aug = [W | W@As | W@Ad]
    waugs = []
    for l in range(3):
        As = np.zeros((HC, H), np.float32)
        Ad = np.zeros((HC, H), np.float32)
        for h in range(H):
            As[h * Ch:(h + 1) * Ch, h] = a_srcs[l][h]
            Ad[h * Ch:(h + 1) * Ch, h] = a_dsts[l][h]
        W = Ws[l]
        waugs.append(np.concatenate([W, W @ As, W @ Ad], axis=1))
    waug = np.stack(waugs, 0).astype(ml_dtypes.bfloat16)  # [3, 128, AUGW]

    # edges + self loops (incl. pad nodes, so every row has >=1 edge)
    src_all = np.concatenate([edge_index[0].astype(np.int64),
                              np.arange(npad, dtype=np.int64)])
    dst_all = np.concatenate([edge_index[1].astype(np.int64),
                              np.arange(npad, dtype=np.int64)])
    (blk_of_tile, start_t, stop_t, bufcol, n_lo, Tpad,
     srcidxs, dlocs, dlocTs) = _prep_edges(src_all, dst_all, per, nb, npad)

    xpad = np.zeros((npad, HC), np.float32)
    xpad[:n] = x
    iota = np.tile(np.arange(P, dtype=np.float32)[None, :], (P, 4))

    batchf_full = np.full(npad, -1.0, np.float32)
    batchf_full[:n] = batch.astype(np.float32)

    nc = _build(npad, Tpad, blk_of_tile, start_t, stop_t, bufcol, n_lo)

    in_maps = []
    for c in range(NCORES):
        sl = slice(c * per, (c + 1) * per)
        in_maps.append({
            "xT": np.ascontiguousarray(xpad[sl].T).astype(ml_dtypes.bfloat16),
            "waug": waug,
            "wh": Wh,
            "iota": iota,
            "iotap": np.arange(P, dtype=np.float32)[:, None].astype(
                ml_dtypes.bfloat16),
            "ident": np.eye(P, dtype=np.float32),
            "srcidx": srcidxs[c],
            "dloc": dlocs[c],
            "iotarep": np.tile(np.arange(P, dtype=np.float32)[None, :],
                               (P, CT)).astype(ml_dtypes.bfloat16),
            "dlocT": dlocTs[c],
            "batchf": np.ascontiguousarray(
                batchf_full[sl].reshape(nb, P).T),
            })
    return nc, in_maps


def run_gat(x, Ws, a_srcs, a_dsts, biases, Wh, bh, edge_index, batch):
    nc, in_maps = prepare(x, Ws, a_srcs, a_dsts, biases, Wh, bh,
                          edge_index, batch)
    res = run_bass_kernel_spmd(nc, in_maps, list(range(NCORES)))
    global LAST_EXEC_NS
    LAST_EXEC_NS = getattr(res, "exec_time_ns", None)
    logits = np.zeros((GRAPHS, OUT), np.float32)
    for c in range(NCORES):
        logits += res.results[c]["out"]
    return logits + bh


def kernel(**inputs):
    return np.asarray(run_gat(
        inputs["x"], inputs["Ws"], inputs["a_srcs"], inputs["a_dsts"],
        inputs["biases"], inputs["Wh"], inputs["bh"], inputs["edge_index"],
        inputs["batch"]), np.float32)
